# revision 1
# baseline (speedup 1.0000x reference)
"""Per-seq tail-chunk variant: base (tiles 0-7) loaded per head-pair; tail
tiles 8-15 grouped per (seq, chunk) across all 4 head-pairs so each cond DMA
is ~1MB and a sequence needs only 4 base + 4 cond DMAs (vs 20 before).

base kvb[s, hp, p, col] (4112 cols):  head hi block at hi*2056:
    [0,1024) K tiles 0-7, [1024, 2056) V tiles 0-7 (129 cols each)
tail kvt[s, c, p, col] (4112 cols), chunk c covers tiles {8+2c, 9+2c}:
    pair hp at hp*1028, head hi at +hi*514: [0,256) K, [256,514) V
Chunk c loaded iff context_len > 1024 + 256c; stale tail data neutralized by
on-device P-masking of tiles 8-15 and startup memset (NaN protection).
"""

import numpy as np

B = 64
H = 32
HK = 8
G = H // HK
D = 128
BS = 16
MAX_CTX = 2048
NCORES = 8
SPC = B // NCORES
HP = HK // 2
NT = MAX_CTX // 128
VW = D + 1
FREE_TOT = MAX_CTX + NT * VW   # 4112 (old per-head layout width)
HBLK = 8 * 128 + 8 * VW        # 2056 per-head block in base
CHW = 2 * 128 + 2 * VW         # 514 per-head section per chunk
PCH = 2 * CHW                  # 1028 per-pair section per chunk
TCH = 4 * PCH                  # 4112 per-chunk region (all pairs)
NCHUNK = 4
SCALE = 0.08838834764831845

_cached_nc = None


def _base_cols(hi, j):
    base = hi * HBLK
    return base + j * 128, base + 1024 + j * VW


def _tail_cols(hp, hi, j):
    c = (j - 8) // 2
    i = (j - 8) % 2
    base = c * TCH + hp * PCH + hi * CHW
    return base + i * 128, base + 256 + i * VW


def _build_nc(reps=1):
    from contextlib import nullcontext

    from concourse import bacc, mybir, tile

    f32 = mybir.dt.float32
    bf16 = mybir.dt.bfloat16
    i32 = mybir.dt.int32
    nc = bacc.Bacc(
        "TRN2",
        target_bir_lowering=False,
        debug=False,
        enable_asserts=False,
        num_devices=NCORES,
    )
    kvb = nc.dram_tensor("kvb", (SPC, HP, 128, 4112), bf16, kind="ExternalInput")
    kvt = nc.dram_tensor("kvt", (SPC, NCHUNK, 128, TCH), bf16, kind="ExternalInput")
    qt = nc.dram_tensor("qt", (128, SPC * HK * G), bf16, kind="ExternalInput")
    offs = nc.dram_tensor("offs", (1, SPC * NCHUNK), i32, kind="ExternalInput")
    msk = nc.dram_tensor("msk", (128, SPC * NT * G // 2), bf16, kind="ExternalInput")
    out = nc.dram_tensor("out", (SPC, HK, G, D), f32, kind="ExternalOutput")

    with tile.TileContext(nc) as tc:
        with (
            tc.tile_pool(name="const", bufs=1) as constp,
            tc.tile_pool(name="kvbp", bufs=8) as kvbp,
            tc.tile_pool(name="kvtp", bufs=2) as kvtp,
            tc.tile_pool(name="pp", bufs=6) as pp,
            tc.tile_pool(name="oseq", bufs=1) as oseqp,
            tc.tile_pool(name="op", bufs=8) as op,
            tc.tile_pool(name="ps_s", bufs=4, space="PSUM") as ps_sp,
            tc.tile_pool(name="ps_o", bufs=3, space="PSUM") as ps_op,
        ):
            qt_sb = constp.tile([128, SPC * HK * G], bf16)
            nc.sync.dma_start(out=qt_sb[:], in_=qt[:])
            msk_sb = constp.tile([128, SPC * NT * G // 2], bf16)
            nc.sync.dma_start(out=msk_sb[:], in_=msk[:])
            offs_sb = constp.tile([1, SPC * NCHUNK], i32)
            nc.sync.dma_start(out=offs_sb[:], in_=offs[:])

            _, off_vals = nc.values_load_multi_w_load_instructions(
                offs_sb[0:1, :],
                engines=[mybir.EngineType.SP],
                min_val=0,
                max_val=1,
                skip_runtime_bounds_check=True,
            )

            for _i in range(2):
                kvt_init = kvtp.tile([128, NCHUNK * TCH], bf16, tag="kvt")
                nc.gpsimd.memset(kvt_init[:], 0.0)

            loop = tc.For_i(0, reps, 1) if reps > 1 else nullcontext()
            with loop:
                o_all = oseqp.tile([G, SPC, HK, D], f32)
                for s in range(SPC):
                    kvt_sb = kvtp.tile([128, NCHUNK * TCH], bf16, tag="kvt")
                    for c in range(NCHUNK):
                        nc.sync.dma_start(
                            out=kvt_sb[:, c * TCH:(c + 1) * TCH],
                            in_=kvt[s, c],
                            cond=off_vals[s * NCHUNK + c],
                        )
                    for hp in range(HP):
                        kvb_sb = kvbp.tile([128, 4112], bf16)
                        nc.sync.dma_start(out=kvb_sb[:], in_=kvb[s, hp])
                        for hi in range(2):
                            h = 2 * hp + hi

                            ps_s = ps_sp.tile([128, NT * G], f32)
                            qcol = (s * HK + h) * G
                            for j in range(NT):
                                if j < 8:
                                    kcol, _ = _base_cols(hi, j)
                                    ksrc = kvb_sb[:, kcol:kcol + 128]
                                else:
                                    kcol, _ = _tail_cols(hp, hi, j)
                                    ksrc = kvt_sb[:, kcol:kcol + 128]
                                nc.tensor.matmul(
                                    ps_s[:, j * G:(j + 1) * G],
                                    ksrc,
                                    qt_sb[:, qcol:qcol + G],
                                    start=True,
                                    stop=True,
                                )

                            p_sb = pp.tile([128, NT * G], bf16)
                            nc.scalar.activation(
                                p_sb[:],
                                ps_s[:],
                                mybir.ActivationFunctionType.Exp,
                                scale=SCALE,
                            )
                            nc.vector.scalar_tensor_tensor(
                                p_sb[:, NT * G // 2:],
                                p_sb[:, NT * G // 2:],
                                1.0,
                                msk_sb[:, s * (NT * G // 2):(s + 1) * (NT * G // 2)],
                                op0=mybir.AluOpType.mult,
                                op1=mybir.AluOpType.mult,
                            )

                            ps_o = ps_op.tile([G, VW], f32)
                            for j in range(NT):
                                if j < 8:
                                    _, vcol = _base_cols(hi, j)
                                    vsrc = kvb_sb[:, vcol:vcol + VW]
                                else:
                                    _, vcol = _tail_cols(hp, hi, j)
                                    vsrc = kvt_sb[:, vcol:vcol + VW]
                                nc.tensor.matmul(
                                    ps_o[:],
                                    p_sb[:, j * G:(j + 1) * G],
                                    vsrc,
                                    start=(j == 0),
                                    stop=(j == NT - 1),
                                )

                            recip = op.tile([G, 1], f32)
                            nc.vector.reciprocal(recip[:], ps_o[:, D:D + 1])
                            nc.vector.tensor_scalar_mul(
                                o_all[:, s, h, :], ps_o[:, 0:D], recip[:]
                            )
                nc.scalar.dma_start(
                    out=out.rearrange("s h g d -> g s h d"), in_=o_all[:]
                )

    nc.compile()
    return nc


def get_nc():
    global _cached_nc
    if _cached_nc is None:
        _cached_nc = _build_nc()
    return _cached_nc


def _to_bf16(a):
    import ml_dtypes
    u = np.ascontiguousarray(a, np.float32).view(np.uint32)
    r = ((u >> 16) & np.uint32(1)) + np.uint32(0x7FFF)
    return ((u + r) >> 16).astype(np.uint16).view(ml_dtypes.bfloat16)


def prepare_in_maps(q, k, v, k_cache, v_cache, slot_mapping, block_tables,
                    context_lens):
    import ml_dtypes
    bf = ml_dtypes.bfloat16

    q = np.asarray(q, np.float32)
    k = np.asarray(k, np.float32)
    v = np.asarray(v, np.float32)
    k_cache = np.asarray(k_cache, np.float32)
    v_cache = np.asarray(v_cache, np.float32)
    slot_mapping = np.asarray(slot_mapping, np.int64)
    block_tables = np.asarray(block_tables, np.int64)
    context_lens = np.asarray(context_lens, np.int64)

    nb, bs, hk, d = k_cache.shape
    S = block_tables.shape[1] * bs

    kc = k_cache.reshape(nb * bs, hk, d).copy()
    vc = v_cache.reshape(nb * bs, hk, d).copy()
    kc[slot_mapping] = k
    vc[slot_mapping] = v

    t = np.arange(S)
    flat = block_tables[:, t // bs] * bs + t % bs
    keys = _to_bf16(kc[flat])
    vals = _to_bf16(vc[flat])
    del kc, vc

    mask01 = (t[None, :] < context_lens[:, None])
    vals[~mask01] = bf(0.0)
    maskf = mask01.astype(bf)

    qt_all = _to_bf16(q)

    # base column mapping: (hi, old FREE_TOT col) for each of the 4112 cols
    b_head = np.empty(4112, np.int64)
    b_col = np.empty(4112, np.int64)
    for hi in range(2):
        for j in range(8):
            kcol, vcol = _base_cols(hi, j)
            b_head[kcol:kcol + 128] = hi
            b_col[kcol:kcol + 128] = j * 128 + np.arange(128)
            b_head[vcol:vcol + VW] = hi
            b_col[vcol:vcol + VW] = MAX_CTX + j * VW + np.arange(VW)

    # tail mapping: (full head, old col) for each (chunk, col in TCH)
    t_head = np.empty((NCHUNK, TCH), np.int64)
    t_col = np.empty((NCHUNK, TCH), np.int64)
    for c in range(NCHUNK):
        for hp in range(HP):
            for hi in range(2):
                for i in range(2):
                    j = 8 + 2 * c + i
                    kcol = hp * PCH + hi * CHW + i * 128
                    vcol = hp * PCH + hi * CHW + 256 + i * VW
                    t_head[c, kcol:kcol + 128] = 2 * hp + hi
                    t_col[c, kcol:kcol + 128] = j * 128 + np.arange(128)
                    t_head[c, vcol:vcol + VW] = 2 * hp + hi
                    t_col[c, vcol:vcol + VW] = MAX_CTX + j * VW + np.arange(VW)

    in_maps = []
    for m in range(NCORES):
        sl = slice(m * SPC, (m + 1) * SPC)
        ks = keys[sl]
        vs = vals[sl]
        mk = maskf[sl]
        lens = context_lens[m * SPC:(m + 1) * SPC]

        kv_old = np.empty((SPC, HK, 128, FREE_TOT), bf)
        kv_old[:, :, :, :MAX_CTX] = ks.transpose(0, 2, 3, 1)
        vp = kv_old[:, :, :, MAX_CTX:].reshape(SPC, HK, 128, NT, VW)
        vt = vs.transpose(0, 2, 1, 3).reshape(SPC, HK, NT, 128, D)
        vp[..., :D] = vt.transpose(0, 1, 3, 2, 4)
        mcol = mk.reshape(SPC, NT, 128).transpose(0, 2, 1)
        vp[..., D] = mcol[:, None, :, :]

        pairs = kv_old.reshape(SPC, HP, 2, 128, FREE_TOT).transpose(0, 1, 3, 2, 4)
        kvb_host = np.ascontiguousarray(pairs[:, :, :, b_head, b_col])

        kv_heads = kv_old.transpose(0, 2, 1, 3)          # (SPC, 128, HK, FREE)
        kvt_host = np.ascontiguousarray(
            kv_heads[:, :, t_head, t_col].transpose(0, 2, 1, 3))  # (SPC,NCHUNK,128,TCH)

        offs_host = np.empty((1, SPC * NCHUNK), np.int32)
        for s in range(SPC):
            for c in range(NCHUNK):
                offs_host[0, s * NCHUNK + c] = 1 if lens[s] > 1024 + 256 * c else 0

        pos = (np.arange(8)[None, :] + 8) * 128 + np.arange(128)[:, None]
        mtail = (pos[None, :, :] < lens[:, None, None])
        msk_host = np.ascontiguousarray(
            np.repeat(mtail[..., None], G, axis=-1)
            .reshape(SPC, 128, NT * G // 2)
            .transpose(1, 0, 2)
            .reshape(128, SPC * NT * G // 2)).astype(bf)

        qt_host = np.ascontiguousarray(
            qt_all[sl].reshape(SPC, HK, G, D).transpose(3, 0, 1, 2)
            .reshape(128, SPC * HK * G)
        )
        in_maps.append({"kvb": kvb_host, "kvt": kvt_host, "qt": qt_host,
                        "offs": offs_host, "msk": msk_host})
    return in_maps


def run_on_hw(in_maps, trace=False, **kwargs):
    from concourse import bass_utils
    from concourse.bass_interp import get_hw_module

    nc = get_nc()
    old_m = nc.m
    nc.m = get_hw_module(nc.m)
    try:
        return bass_utils.run_bass_kernel_spmd(
            nc, in_maps, core_ids=list(range(NCORES)), trace=trace, **kwargs
        )
    finally:
        nc.m = old_m


def kernel(q, k, v, k_cache, v_cache, slot_mapping, block_tables, context_lens):
    in_maps = prepare_in_maps(q, k, v, k_cache, v_cache, slot_mapping,
                              block_tables, context_lens)
    res = run_on_hw(in_maps, trace=False)
    outs = [r["out"].reshape(SPC, H * D) for r in res.results]
    return np.concatenate(outs, axis=0).astype(np.float32, copy=False)



# revision 4
# speedup vs baseline: 2.0589x; 2.0589x over previous
"""Paged-attention decode, fp8(e3m4) KV variant.

Per-core layout (all KV data fp8 e3m4, 1B/elem; q/P/mask bf16):
  kvb[s, p, c]  base (tiles 0-7, always loaded): head h block at h*2056,
      tile j at +j*257: [0,128) K cols (partition=d), [128,257) V cols
      (partition=pos, col 128 = validity mask for the denominator).
  kvt[s, c, p, col]  tail chunk c covers tile 8+c: head h at h*257,
      same [K 128 | V 129] split. Loaded iff context_len > 1024+128c.
Sequences are permuted on the host so every core's total loaded bytes are
near-equal (greedy balance); outputs are inverse-permuted.
Stale tail data is neutralized by on-device P-masking of tiles 8-15 and a
startup memset (NaN protection). Scores: K(e3m4) stationary x q(bf16)
moving; PV: P(bf16) stationary x V(e3m4) moving (mixed-dtype matmuls are
HW-exact to fp22).
"""

import numpy as np

B = 64
H = 32
HK = 8
G = H // HK
D = 128
BS = 16
MAX_CTX = 2048
NCORES = 8
SPC = B // NCORES
NT = MAX_CTX // 128
VW = D + 1
TPH = 257                # K(128) + V(129) cols per (head, tile)
HBB = 8 * TPH            # per-head base block (tiles 0-7) = 2056
BASEW = HK * HBB         # base cols per seq = 16448
CHW = HK * TPH           # per-chunk cols (all heads, 1 tile) = 2056
NCHUNK = 8
SCALE = 0.08838834764831845

_cached_nc = None
_last_perm = None


def _build_nc(reps=1):
    from contextlib import nullcontext

    from concourse import bacc, mybir, tile

    f32 = mybir.dt.float32
    bf16 = mybir.dt.bfloat16
    f8e3 = mybir.dt.float8e3
    i32 = mybir.dt.int32
    nc = bacc.Bacc(
        "TRN2",
        target_bir_lowering=False,
        debug=False,
        enable_asserts=False,
        num_devices=NCORES,
    )
    kvb = nc.dram_tensor("kvb", (SPC, 128, BASEW), f8e3, kind="ExternalInput")
    kvt = nc.dram_tensor("kvt", (SPC, NCHUNK, 128, CHW), f8e3, kind="ExternalInput")
    qt = nc.dram_tensor("qt", (128, SPC * HK * G), bf16, kind="ExternalInput")
    offs = nc.dram_tensor("offs", (1, SPC * NCHUNK), i32, kind="ExternalInput")
    msk = nc.dram_tensor("msk", (128, SPC * NT * G // 2), bf16, kind="ExternalInput")
    out = nc.dram_tensor("out", (SPC, HK, G, D), f32, kind="ExternalOutput")

    with tile.TileContext(nc) as tc:
        with (
            tc.tile_pool(name="const", bufs=1) as constp,
            tc.tile_pool(name="kvbp", bufs=2) as kvbp,
            tc.tile_pool(name="kvtp", bufs=2) as kvtp,
            tc.tile_pool(name="pp", bufs=6) as pp,
            tc.tile_pool(name="oseq", bufs=1) as oseqp,
            tc.tile_pool(name="op", bufs=8) as op,
            tc.tile_pool(name="ps_s", bufs=4, space="PSUM") as ps_sp,
            tc.tile_pool(name="ps_o", bufs=3, space="PSUM") as ps_op,
        ):
            qt_sb = constp.tile([128, SPC * HK * G], bf16)
            nc.sync.dma_start(out=qt_sb[:], in_=qt[:])
            msk_sb = constp.tile([128, SPC * NT * G // 2], bf16)
            nc.sync.dma_start(out=msk_sb[:], in_=msk[:])
            offs_sb = constp.tile([1, SPC * NCHUNK], i32)
            nc.sync.dma_start(out=offs_sb[:], in_=offs[:])

            for _i in range(2):
                kvt_init = kvtp.tile([128, NCHUNK * CHW], f8e3, tag="kvt")
                nc.gpsimd.memset(kvt_init[:], 0.0)

            loop = tc.For_i(0, reps, 1) if reps > 1 else nullcontext()
            with loop:
                o_all = oseqp.tile([G, SPC, HK, D], f32)
                for s in range(SPC):
                    kvb_sb = kvbp.tile([128, BASEW], f8e3)
                    nc.sync.dma_start(out=kvb_sb[:], in_=kvb[s])
                    _, off_vals = nc.values_load_multi_w_load_instructions(
                        offs_sb[0:1, s * NCHUNK:(s + 1) * NCHUNK],
                        engines=[mybir.EngineType.SP],
                        min_val=0,
                        max_val=1,
                        skip_runtime_bounds_check=True,
                    )
                    kvt_sb = kvtp.tile([128, NCHUNK * CHW], f8e3, tag="kvt")
                    for c in range(NCHUNK):
                        nc.sync.dma_start(
                            out=kvt_sb[:, c * CHW:(c + 1) * CHW],
                            in_=kvt[s, c],
                            cond=off_vals[c],
                        )
                    for h in range(HK):
                        ps_s = ps_sp.tile([128, NT * G], f32)
                        qcol = (s * HK + h) * G
                        for j in range(NT):
                            if j < 8:
                                kcol = h * HBB + j * TPH
                                ksrc = kvb_sb[:, kcol:kcol + 128]
                            else:
                                kcol = (j - 8) * CHW + h * TPH
                                ksrc = kvt_sb[:, kcol:kcol + 128]
                            nc.tensor.matmul(
                                ps_s[:, j * G:(j + 1) * G],
                                ksrc,
                                qt_sb[:, qcol:qcol + G],
                                start=True,
                                stop=True,
                            )

                        p_sb = pp.tile([128, NT * G], bf16)
                        nc.scalar.activation(
                            p_sb[:],
                            ps_s[:],
                            mybir.ActivationFunctionType.Exp,
                            scale=SCALE,
                        )
                        nc.vector.scalar_tensor_tensor(
                            p_sb[:, NT * G // 2:],
                            p_sb[:, NT * G // 2:],
                            1.0,
                            msk_sb[:, s * (NT * G // 2):(s + 1) * (NT * G // 2)],
                            op0=mybir.AluOpType.mult,
                            op1=mybir.AluOpType.mult,
                        )

                        ps_o = ps_op.tile([G, VW], f32)
                        for j in range(NT):
                            if j < 8:
                                vcol = h * HBB + j * TPH + 128
                                vsrc = kvb_sb[:, vcol:vcol + VW]
                            else:
                                vcol = (j - 8) * CHW + h * TPH + 128
                                vsrc = kvt_sb[:, vcol:vcol + VW]
                            nc.tensor.matmul(
                                ps_o[:],
                                p_sb[:, j * G:(j + 1) * G],
                                vsrc,
                                start=(j == 0),
                                stop=(j == NT - 1),
                            )

                        recip = op.tile([G, 1], f32)
                        nc.vector.reciprocal(recip[:], ps_o[:, D:D + 1])
                        nc.vector.tensor_scalar_mul(
                            o_all[:, s, h, :], ps_o[:, 0:D], recip[:]
                        )
                nc.scalar.dma_start(
                    out=out.rearrange("s h g d -> g s h d"), in_=o_all[:]
                )

    nc.compile()
    return nc


def get_nc():
    global _cached_nc
    if _cached_nc is None:
        _cached_nc = _build_nc()
    return _cached_nc


def _balance_perm(context_lens):
    """Greedy assignment of seqs to cores equalizing loaded bytes.
    Returns perm: perm[c*SPC + i] = original seq index."""
    lens = np.asarray(context_lens, np.int64)
    loaded = 1024 + 128 * np.ceil(np.maximum(lens - 1024, 0) / 128).astype(np.int64)
    order = np.argsort(-loaded, kind="stable")
    coreload = np.zeros(NCORES, np.int64)
    corecnt = np.zeros(NCORES, np.int64)
    assign = [[] for _ in range(NCORES)]
    for i in order:
        c = int(np.argmin(np.where(corecnt < SPC, coreload, np.iinfo(np.int64).max)))
        coreload[c] += loaded[i]
        corecnt[c] += 1
        assign[c].append(int(i))
    return np.array([i for a in assign for i in a], np.int64)


def prepare_in_maps(q, k, v, k_cache, v_cache, slot_mapping, block_tables,
                    context_lens):
    import ml_dtypes
    global _last_perm
    bf = ml_dtypes.bfloat16
    e3 = ml_dtypes.float8_e3m4

    q = np.asarray(q, np.float32)
    k = np.asarray(k, np.float32)
    v = np.asarray(v, np.float32)
    k_cache = np.asarray(k_cache, np.float32)
    v_cache = np.asarray(v_cache, np.float32)
    slot_mapping = np.asarray(slot_mapping, np.int64)
    block_tables = np.asarray(block_tables, np.int64)
    context_lens = np.asarray(context_lens, np.int64)

    nb, bs, hk, d = k_cache.shape
    S = block_tables.shape[1] * bs

    perm = _balance_perm(context_lens)
    _last_perm = perm

    kc = k_cache.reshape(nb * bs, hk, d)
    vc = v_cache.reshape(nb * bs, hk, d).copy()
    kc_w = kc.copy()
    kc_w[slot_mapping] = k
    vc[slot_mapping] = v

    t = np.arange(S)
    flat = block_tables[:, t // bs] * bs + t % bs      # [B, S]
    flat = flat[perm]                                  # permuted seq order
    keys = kc_w[flat].astype(e3)                       # [B, S, HK, D]
    vals = vc[flat]
    lens_p = context_lens[perm]

    mask01 = (t[None, :] < lens_p[:, None])            # [B, S]
    vals[~mask01] = 0.0
    vals_e = vals.astype(e3)
    del vals

    qt_all = q[perm].astype(bf)

    in_maps = []
    for m in range(NCORES):
        sl = slice(m * SPC, (m + 1) * SPC)
        ks = keys[sl]                                  # [SPC, S, HK, D] e3
        vs = vals_e[sl]
        mk = mask01[sl]
        lens = lens_p[sl]

        # assemble [SPC, 128, HK, NT, 257]
        A = np.empty((SPC, 128, HK, NT, TPH), e3)
        # K: partition = d, free = pos-within-tile
        A[..., :128] = ks.reshape(SPC, NT, 128, HK, D).transpose(0, 4, 3, 1, 2)
        # V: partition = pos-within-tile, free = d
        A[..., 128:256] = vs.reshape(SPC, NT, 128, HK, D).transpose(0, 2, 3, 1, 4)
        # mask column for the denominator
        mcol = mk.reshape(SPC, NT, 128).transpose(0, 2, 1).astype(e3)
        A[..., 256] = mcol[:, :, None, :]

        kvb_host = np.ascontiguousarray(
            A[:, :, :, :8, :].reshape(SPC, 128, BASEW))
        kvt_host = np.ascontiguousarray(
            A[:, :, :, 8:, :].transpose(0, 3, 1, 2, 4).reshape(
                SPC, NCHUNK, 128, CHW))

        offs_host = np.empty((1, SPC * NCHUNK), np.int32)
        for s in range(SPC):
            for c in range(NCHUNK):
                offs_host[0, s * NCHUNK + c] = (
                    1 if lens[s] > 1024 + 128 * c else 0)

        pos = (np.arange(8)[None, :] + 8) * 128 + np.arange(128)[:, None]
        mtail = (pos[None, :, :] < lens[:, None, None])     # [SPC, 128, 8]
        msk_host = np.ascontiguousarray(
            np.repeat(mtail[..., None], G, axis=-1)
            .reshape(SPC, 128, NT * G // 2)
            .transpose(1, 0, 2)
            .reshape(128, SPC * NT * G // 2)).astype(bf)

        qt_host = np.ascontiguousarray(
            qt_all[sl].reshape(SPC, HK, G, D).transpose(3, 0, 1, 2)
            .reshape(128, SPC * HK * G)
        )
        in_maps.append({"kvb": kvb_host, "kvt": kvt_host, "qt": qt_host,
                        "offs": offs_host, "msk": msk_host})
    return in_maps


def run_on_hw(in_maps, trace=False, **kwargs):
    from concourse import bass_utils
    from concourse.bass_interp import get_hw_module

    nc = get_nc()
    old_m = nc.m
    nc.m = get_hw_module(nc.m)
    try:
        return bass_utils.run_bass_kernel_spmd(
            nc, in_maps, core_ids=list(range(NCORES)), trace=trace, **kwargs
        )
    finally:
        nc.m = old_m


def kernel(q, k, v, k_cache, v_cache, slot_mapping, block_tables, context_lens):
    in_maps = prepare_in_maps(q, k, v, k_cache, v_cache, slot_mapping,
                              block_tables, context_lens)
    res = run_on_hw(in_maps, trace=False)
    outs = [r["out"].reshape(SPC, H * D) for r in res.results]
    permuted = np.concatenate(outs, axis=0)
    full = np.empty_like(permuted)
    full[_last_perm] = permuted
    return full.astype(np.float32, copy=False)


# revision 10
# speedup vs baseline: 2.0808x; 1.0107x over previous
"""Paged-attention decode, fp8(e3m4) KV variant.

Per-core layout (all KV data fp8 e3m4, 1B/elem; q/P/mask bf16):
  kvb[s, p, c]  base (tiles 0-7, always loaded): head h block at h*2056,
      tile j at +j*257: [0,128) K cols (partition=d), [128,257) V cols
      (partition=pos, col 128 = validity mask for the denominator).
  kvt[s, c, p, col]  tail chunk c covers tile 8+c: head h at h*257,
      same [K 128 | V 129] split. Loaded iff context_len > 1024+128c.
Sequences are permuted on the host so every core's total loaded bytes are
near-equal (greedy balance); outputs are inverse-permuted.
Stale tail data is neutralized by on-device P-masking of tiles 8-15 and a
startup memset (NaN protection). Scores: K(e3m4) stationary x q(bf16)
moving; PV: P(bf16) stationary x V(e3m4) moving (mixed-dtype matmuls are
HW-exact to fp22).
"""

import numpy as np

B = 64
H = 32
HK = 8
G = H // HK
D = 128
BS = 16
MAX_CTX = 2048
NCORES = 8
SPC = B // NCORES
NT = MAX_CTX // 128
VW = D + 1
TPH = 257                # K(128) + V(129) cols per (head, tile)
HBB = 8 * TPH            # per-head base block (tiles 0-7) = 2056
BASEW = HK * HBB         # base cols per seq = 16448
CHW = HK * TPH           # per-chunk cols (all heads, 1 tile) = 2056
NCHUNK = 8
SCALE = 0.08838834764831845

_cached_nc = None
_last_perm = None


def _build_nc(reps=1):
    from contextlib import nullcontext

    from concourse import bacc, mybir, tile

    f32 = mybir.dt.float32
    bf16 = mybir.dt.bfloat16
    f8e3 = mybir.dt.float8e3
    i32 = mybir.dt.int32
    nc = bacc.Bacc(
        "TRN2",
        target_bir_lowering=False,
        debug=False,
        enable_asserts=False,
        num_devices=NCORES,
    )
    kvb = nc.dram_tensor("kvb", (SPC, 2, 128, BASEW // 2), f8e3,
                         kind="ExternalInput")
    kvt = nc.dram_tensor("kvt", (SPC, NCHUNK, 128, CHW), f8e3, kind="ExternalInput")
    qt = nc.dram_tensor("qt", (128, SPC * HK * G), bf16, kind="ExternalInput")
    offs = nc.dram_tensor("offs", (1, SPC * NCHUNK), i32, kind="ExternalInput")
    msk = nc.dram_tensor("msk", (128, SPC * NT * G // 2), bf16, kind="ExternalInput")
    out = nc.dram_tensor("out", (SPC, HK, G, D), f32, kind="ExternalOutput")

    with tile.TileContext(nc) as tc:
        with (
            tc.tile_pool(name="const", bufs=1) as constp,
            tc.tile_pool(name="kvbp", bufs=3) as kvbp,
            tc.tile_pool(name="kvbp2", bufs=3) as kvbp2,
            tc.tile_pool(name="kvtp", bufs=3) as kvtp,
            tc.tile_pool(name="pp", bufs=6) as pp,
            tc.tile_pool(name="oseq", bufs=1) as oseqp,
            tc.tile_pool(name="op", bufs=8) as op,
            tc.tile_pool(name="ps_s", bufs=4, space="PSUM") as ps_sp,
            tc.tile_pool(name="ps_o", bufs=3, space="PSUM") as ps_op,
        ):
            qt_sb = constp.tile([128, SPC * HK * G], bf16)
            nc.sync.dma_start(out=qt_sb[:], in_=qt[:])
            msk_sb = constp.tile([128, SPC * NT * G // 2], bf16)
            nc.sync.dma_start(out=msk_sb[:], in_=msk[:])
            offs_sb = constp.tile([1, SPC * NCHUNK], i32)
            nc.sync.dma_start(out=offs_sb[:], in_=offs[:])

            for _i in range(3):
                kvt_init = kvtp.tile([128, NCHUNK * CHW], f8e3, tag="kvt")
                nc.gpsimd.memset(kvt_init[:], 0.0)

            loop = tc.For_i(0, reps, 1) if reps > 1 else nullcontext()
            with loop:
                o_all = oseqp.tile([G, SPC, HK, D], f32)
                for s in range(SPC):
                    kvb_a = kvbp.tile([128, BASEW // 2], f8e3)
                    nc.sync.dma_start(out=kvb_a[:], in_=kvb[s, 0])
                    _, off_vals = nc.values_load_multi_w_load_instructions(
                        offs_sb[0:1, s * NCHUNK:(s + 1) * NCHUNK],
                        engines=[mybir.EngineType.SP],
                        min_val=0,
                        max_val=1,
                        skip_runtime_bounds_check=True,
                    )
                    kvt_sb = kvtp.tile([128, NCHUNK * CHW], f8e3, tag="kvt")
                    for c in range(NCHUNK):
                        nc.sync.dma_start(
                            out=kvt_sb[:, c * CHW:(c + 1) * CHW],
                            in_=kvt[s, c],
                            cond=off_vals[c],
                        )
                    kvb_b = kvbp2.tile([128, BASEW // 2], f8e3)
                    nc.sync.dma_start(out=kvb_b[:], in_=kvb[s, 1])
                    for h in range(HK):
                        kvb_sb = kvb_a if h < 4 else kvb_b
                        hh = h % 4
                        ps_s = ps_sp.tile([128, NT * G], f32)
                        qcol = (s * HK + h) * G
                        for j in range(NT):
                            if j < 8:
                                kcol = hh * HBB + j * TPH
                                ksrc = kvb_sb[:, kcol:kcol + 128]
                            else:
                                kcol = (j - 8) * CHW + h * TPH
                                ksrc = kvt_sb[:, kcol:kcol + 128]
                            nc.tensor.matmul(
                                ps_s[:, j * G:(j + 1) * G],
                                ksrc,
                                qt_sb[:, qcol:qcol + G],
                                start=True,
                                stop=True,
                            )

                        p_sb = pp.tile([128, NT * G], bf16)
                        nc.scalar.activation(
                            p_sb[:],
                            ps_s[:],
                            mybir.ActivationFunctionType.Exp,
                            scale=SCALE,
                        )
                        nc.vector.scalar_tensor_tensor(
                            p_sb[:, NT * G // 2:],
                            p_sb[:, NT * G // 2:],
                            1.0,
                            msk_sb[:, s * (NT * G // 2):(s + 1) * (NT * G // 2)],
                            op0=mybir.AluOpType.mult,
                            op1=mybir.AluOpType.mult,
                        )

                        ps_o = ps_op.tile([G, VW], f32)
                        for j in range(NT):
                            if j < 8:
                                vcol = hh * HBB + j * TPH + 128
                                vsrc = kvb_sb[:, vcol:vcol + VW]
                            else:
                                vcol = (j - 8) * CHW + h * TPH + 128
                                vsrc = kvt_sb[:, vcol:vcol + VW]
                            nc.tensor.matmul(
                                ps_o[:],
                                p_sb[:, j * G:(j + 1) * G],
                                vsrc,
                                start=(j == 0),
                                stop=(j == NT - 1),
                            )

                        recip = op.tile([G, 1], f32)
                        nc.vector.reciprocal(recip[:], ps_o[:, D:D + 1])
                        nc.vector.tensor_scalar_mul(
                            o_all[:, s, h, :], ps_o[:, 0:D], recip[:]
                        )
                nc.scalar.dma_start(
                    out=out.rearrange("s h g d -> g s h d"), in_=o_all[:]
                )

    nc.compile()
    return nc


def get_nc():
    global _cached_nc
    if _cached_nc is None:
        _cached_nc = _build_nc()
    return _cached_nc


def _balance_perm(context_lens):
    """Greedy assignment of seqs to cores equalizing loaded bytes.
    Returns perm: perm[c*SPC + i] = original seq index."""
    lens = np.asarray(context_lens, np.int64)
    loaded = 1024 + 128 * np.ceil(np.maximum(lens - 1024, 0) / 128).astype(np.int64)
    order = np.argsort(-loaded, kind="stable")
    coreload = np.zeros(NCORES, np.int64)
    corecnt = np.zeros(NCORES, np.int64)
    assign = [[] for _ in range(NCORES)]
    for i in order:
        c = int(np.argmin(np.where(corecnt < SPC, coreload, np.iinfo(np.int64).max)))
        coreload[c] += loaded[i]
        corecnt[c] += 1
        assign[c].append(int(i))
    return np.array([i for a in assign for i in a], np.int64)


def prepare_in_maps(q, k, v, k_cache, v_cache, slot_mapping, block_tables,
                    context_lens):
    import ml_dtypes
    global _last_perm
    bf = ml_dtypes.bfloat16
    e3 = ml_dtypes.float8_e3m4

    q = np.asarray(q, np.float32)
    k = np.asarray(k, np.float32)
    v = np.asarray(v, np.float32)
    k_cache = np.asarray(k_cache, np.float32)
    v_cache = np.asarray(v_cache, np.float32)
    slot_mapping = np.asarray(slot_mapping, np.int64)
    block_tables = np.asarray(block_tables, np.int64)
    context_lens = np.asarray(context_lens, np.int64)

    nb, bs, hk, d = k_cache.shape
    S = block_tables.shape[1] * bs

    perm = _balance_perm(context_lens)
    _last_perm = perm

    kc = k_cache.reshape(nb * bs, hk, d)
    vc = v_cache.reshape(nb * bs, hk, d).copy()
    kc_w = kc.copy()
    kc_w[slot_mapping] = k
    vc[slot_mapping] = v

    t = np.arange(S)
    flat = block_tables[:, t // bs] * bs + t % bs      # [B, S]
    flat = flat[perm]                                  # permuted seq order
    keys = kc_w[flat].astype(e3)                       # [B, S, HK, D]
    vals = vc[flat]
    lens_p = context_lens[perm]

    mask01 = (t[None, :] < lens_p[:, None])            # [B, S]
    vals[~mask01] = 0.0
    vals_e = vals.astype(e3)
    del vals

    qt_all = q[perm].astype(bf)

    in_maps = []
    for m in range(NCORES):
        sl = slice(m * SPC, (m + 1) * SPC)
        ks = keys[sl]                                  # [SPC, S, HK, D] e3
        vs = vals_e[sl]
        mk = mask01[sl]
        lens = lens_p[sl]

        # assemble [SPC, 128, HK, NT, 257]
        A = np.empty((SPC, 128, HK, NT, TPH), e3)
        # K: partition = d, free = pos-within-tile
        A[..., :128] = ks.reshape(SPC, NT, 128, HK, D).transpose(0, 4, 3, 1, 2)
        # V: partition = pos-within-tile, free = d
        A[..., 128:256] = vs.reshape(SPC, NT, 128, HK, D).transpose(0, 2, 3, 1, 4)
        # mask column for the denominator
        mcol = mk.reshape(SPC, NT, 128).transpose(0, 2, 1).astype(e3)
        A[..., 256] = mcol[:, :, None, :]

        kvb_host = np.ascontiguousarray(
            A[:, :, :, :8, :].reshape(SPC, 128, 2, BASEW // 2)
            .transpose(0, 2, 1, 3))
        kvt_host = np.ascontiguousarray(
            A[:, :, :, 8:, :].transpose(0, 3, 1, 2, 4).reshape(
                SPC, NCHUNK, 128, CHW))

        offs_host = np.empty((1, SPC * NCHUNK), np.int32)
        for s in range(SPC):
            for c in range(NCHUNK):
                offs_host[0, s * NCHUNK + c] = (
                    1 if lens[s] > 1024 + 128 * c else 0)

        pos = (np.arange(8)[None, :] + 8) * 128 + np.arange(128)[:, None]
        mtail = (pos[None, :, :] < lens[:, None, None])     # [SPC, 128, 8]
        msk_host = np.ascontiguousarray(
            np.repeat(mtail[..., None], G, axis=-1)
            .reshape(SPC, 128, NT * G // 2)
            .transpose(1, 0, 2)
            .reshape(128, SPC * NT * G // 2)).astype(bf)

        qt_host = np.ascontiguousarray(
            qt_all[sl].reshape(SPC, HK, G, D).transpose(3, 0, 1, 2)
            .reshape(128, SPC * HK * G)
        )
        in_maps.append({"kvb": kvb_host, "kvt": kvt_host, "qt": qt_host,
                        "offs": offs_host, "msk": msk_host})
    return in_maps


def run_on_hw(in_maps, trace=False, **kwargs):
    from concourse import bass_utils
    from concourse.bass_interp import get_hw_module

    nc = get_nc()
    old_m = nc.m
    nc.m = get_hw_module(nc.m)
    try:
        return bass_utils.run_bass_kernel_spmd(
            nc, in_maps, core_ids=list(range(NCORES)), trace=trace, **kwargs
        )
    finally:
        nc.m = old_m


def kernel(q, k, v, k_cache, v_cache, slot_mapping, block_tables, context_lens):
    in_maps = prepare_in_maps(q, k, v, k_cache, v_cache, slot_mapping,
                              block_tables, context_lens)
    res = run_on_hw(in_maps, trace=False)
    outs = [r["out"].reshape(SPC, H * D) for r in res.results]
    permuted = np.concatenate(outs, axis=0)
    full = np.empty_like(permuted)
    full[_last_perm] = permuted
    return full.astype(np.float32, copy=False)
